# revision 26
# baseline (speedup 1.0000x reference)
"""AVWGCN (adaptive vertex-wise graph convolution) Trainium2 kernel.

Reference computation (per batch b):
  bias = STE @ bias_pool                               [n, o]
  T0 = SC, T1 = R, T2 = 2 R@R - SC                     (Chebyshev, K=3)
  h_k = T_k @ x                                        [n, k, i]
  z   = einsum('nki,dkio->ndo', h, weights_pool)
  out = einsum('ndo,nd->no', z, STE) + bias

Key algebraic restructure: T2 is only used via T2 @ x, so
  h_2 = 2 R @ (R @ x) = 2 R @ h_1 - h_0
which avoids the O(N^3) matmul entirely.

Sharding: data-parallel over batch, 4 batches per core across 8 cores.
All matmul operands are bf16 (fp32 PSUM accumulation): halves the HBM
traffic for the dominant R/SC loads and the LDWEIGHTS cost, at ~3e-3
rel err (gate is 2e-2).

Layout notes (per core):
  - PE matmuls contract the partition dim of both operands, so the graph
    matrices are needed with m (their column index) on partitions; the
    per-core shards of R/SC are uploaded host-pre-swizzled to
    [b, s, p, n] (m = s*128+p) so every stripe DMA is fully linear.
  - h is produced directly in transposed layout hT[(k,i), pos] via
    out = lhsT.T @ rhs with lhsT = x (so no transpose of h needed for z).
  - z psum tiles are [pos=128, (o,d)=1024] halves (d innermost); the STE
    contraction over d runs as ONE custom fused DVE op per half:
    prefix-sum-of-products (scan) whose output AP collapses the d axis
    (stride-0 write), leaving cumulative group ends; a shifted subtract
    per pos-chunk turns those into the final grouped sums.
  - bias is folded into z as an extra contraction row: hT_b carries a
    constant-ones row 64, and W2b carries bias_pool as row 64.  The 2.0
    of the Chebyshev recurrence is folded into W2b rows 0-63 on host.
  - DMA rings (~100GB/s each): SCT stripes ride sync HWDGE, RT stripes
    ride the gpsimd SWDGE, everything else + stores ride scalar HWDGE.
  - tail: grouped sums + output stores are issued per pos-chunk, and the
    last chunk drains at per-512-col granularity, so the drain after the
    final matmul is one half-sized scan + subtract + store.
"""

import sys

sys.path.insert(0, "/opt/trn_rl_repo")

import numpy as np
import ml_dtypes

import concourse.bacc as bacc
import concourse.mybir as mybir
import concourse.tile as tile
from concourse import bass_utils
from concourse import dve_ops as _dv
from concourse.dve_spec import Spec, Src0, Src1, scan, AluOp, lower
from concourse.dve_ops import DveOp, OPS
from concourse.dve_uop import DveOpSpec

F32 = mybir.dt.float32
BF16 = mybir.dt.bfloat16

B, N, DIN, DOUT, CHEB_K, ET = 32, 512, 64, 64, 3, 32
N_CORES = 8
B_PER_CORE = B // N_CORES  # 4
P = 128
S = N // P  # 4 pos-chunks per batch
DO = DOUT * ET  # 2048
HALF_O = DOUT // 2  # 32 o-values per psum half

_cached = {}


def _mulscan_op():
    """out = prefix_sum(in0 * in1) along the free stream (fp32 accum)."""
    if "mulscan" in _cached:
        return _cached["mulscan"]

    def _ref(in0, in1, s0, s1, imm2):
        prod = in0.astype(np.float32) * in1.astype(np.float32)
        flat = np.cumsum(prod.reshape(prod.shape[0], -1), axis=1)
        return flat.reshape(prod.shape).astype(np.float32)

    spec = Spec(body=scan(AluOp.ADD, Src0 * Src1), reference=_ref)
    shas = {}
    for ver in ("v3", "v4"):
        s = DveOpSpec(name="MULSCAN_ANT", opcode=0, uops=lower(spec, ver=ver), rd1_en=True)
        shas[ver] = s.sha(ver)
    op = DveOp("MULSCAN_ANT", spec, subdim=False, uops_sha=shas)
    OPS.append(op)
    _dv._SUB_OPCODE_FOR_NAME[op.name] = _dv._CUSTOM_DVE_ROW_BASE + len(OPS) - 1
    _dv.CUSTOM_DVE_SPECS[op.name] = op.spec
    _cached["mulscan"] = op
    return op


def _build_kernel():
    MULSCAN = _mulscan_op()
    nc = bacc.Bacc("TRN2", target_bir_lowering=False)

    # RT/SCT are the per-core R/SC shards pre-swizzled on host to
    # [b, s, p, n] with m = s*128 + p, so each stripe load is linear.
    RT_d = nc.dram_tensor("RT", [B_PER_CORE, S, P, N], BF16, kind="ExternalInput")
    SCT_d = nc.dram_tensor("SCT", [B_PER_CORE, S, P, N], BF16, kind="ExternalInput")
    # x/STE arrive pre-swizzled to the SBUF layout [p, b, s, i].
    x_d = nc.dram_tensor("x", [P, B_PER_CORE, S, DIN], BF16, kind="ExternalInput")
    STE_d = nc.dram_tensor("STE", [P, B_PER_CORE, S, ET], F32, kind="ExternalInput")
    # W2a: rows ki=0..127 (k=0,1); W2b: rows ki=128..191 (k=2, pre-scaled
    # by the Chebyshev 2.0) + bias row.  columns ordered (o, d).
    W2a_d = nc.dram_tensor("W2a", [P, DO], BF16, kind="ExternalInput")
    W2b_d = nc.dram_tensor("W2b", [DIN + 1, DO], BF16, kind="ExternalInput")
    I128_d = nc.dram_tensor("I128", [P, P], BF16, kind="ExternalInput")
    out_d = nc.dram_tensor("out", [B_PER_CORE, N, DOUT], F32, kind="ExternalOutput")

    with tile.TileContext(nc) as tc:
        with (
            tc.tile_pool(name="const", bufs=1) as cpool,
            tc.tile_pool(name="load", bufs=4) as lpool,
            tc.tile_pool(name="work", bufs=2) as wpool,
            tc.tile_pool(name="psh", bufs=1, space="PSUM") as psh,
            tc.tile_pool(name="psz", bufs=3, space="PSUM") as psz,
        ):
            w2a = cpool.tile([P, DO], BF16)
            w2b = cpool.tile([DIN + 1, DO], BF16)
            ident = cpool.tile([P, P], BF16)
            xall = cpool.tile([P, B_PER_CORE, S, DIN], BF16)
            steall = cpool.tile([P, B_PER_CORE, S, ET], F32)

            # ident on the (otherwise idle at start) gpsimd queue so the
            # warmup chain fires immediately; everything else on scalar.
            nc.gpsimd.dma_start(ident[:], I128_d[:])
            nc.scalar.dma_start(xall[:], x_d[:])
            nc.scalar.dma_start(steall[:], STE_d[:])
            nc.scalar.dma_start(w2a[:], W2a_d[:])
            nc.scalar.dma_start(w2b[:], W2b_d[:])

            # PE warmup: ramp the pstate while the first loads land.
            warm = psh.tile([P, N], F32, tag="ph0")
            for w in range(8):
                nc.tensor.matmul(
                    warm[:, 0:P], ident[:], ident[:],
                    start=(w == 0), stop=(w == 7),
                )

            # The hT_b ones row (bias) is identical across batches: set it
            # once per rotating buffer, outside the batch loop, so no
            # per-batch memset ever sits on a busy queue.
            for _ in range(2):
                hT_b_init = wpool.tile([DIN + 1, N], BF16, tag="hT_b")
                nc.gpsimd.memset(hT_b_init[DIN : DIN + 1, :], 1.0)

            def emit_loads(b):
                RT = lpool.tile([P, S, N], BF16, tag="RT")
                SCT = lpool.tile([P, S, N], BF16, tag="SCT")
                for s in range(S):
                    nc.sync.dma_start(SCT[:, s], SCT_d[b, s])
                for s in range(S):
                    nc.gpsimd.dma_start(RT[:, s], RT_d[b, s])
                return RT, SCT

            def emit_h01(b, RT, SCT):
                xb = xall[:, b]
                hT_a = wpool.tile([P, N], BF16, tag="hT_a")
                ph0 = psh.tile([DIN, N], F32, tag="ph0")
                ph1 = psh.tile([DIN, N], F32, tag="ph1")
                for s in range(S):
                    nc.tensor.matmul(
                        ph0[:], xb[:, s, :], SCT[:, s, :],
                        start=(s == 0), stop=(s == S - 1),
                    )
                for s in range(S):
                    nc.tensor.matmul(
                        ph1[:], xb[:, s, :], RT[:, s, :],
                        start=(s == 0), stop=(s == S - 1),
                    )
                nc.scalar.copy(hT_a[0:DIN, :], ph0[:])
                nc.scalar.copy(hT_a[DIN : 2 * DIN, :], ph1[:])
                ph1n = psh.tile([P, S * DIN], BF16, tag="ph1")
                for c in range(S):
                    nc.tensor.transpose(
                        ph1n[:, DIN * c : DIN * (c + 1)],
                        hT_a[DIN : 2 * DIN, P * c : P * (c + 1)],
                        ident[DIN : 2 * DIN, DIN : 2 * DIN],
                    )
                h1n = wpool.tile([P, S, DIN], BF16, tag="h1n")
                nc.scalar.copy(h1n[:], ph1n[:].rearrange("p (s i) -> p s i", i=DIN))
                return hT_a, h1n

            def emit_h2(b, RT, h1n):
                ph2 = psh.tile([DIN, N], F32, tag="ph0")
                for s in range(S):
                    nc.tensor.matmul(
                        ph2[:], h1n[:, s, :], RT[:, s, :],
                        start=(s == 0), stop=(s == S - 1),
                    )
                hT_b = wpool.tile([DIN + 1, N], BF16, tag="hT_b")
                nc.scalar.copy(hT_b[0:DIN, :], ph2[:])
                return hT_b

            def z_chunk(b, s, hT_a, hT_b, pz_pre):
                fine = b == B_PER_CORE - 1 and s == S - 1
                steb = steall[:, b]
                E = wpool.tile([P, DOUT], F32, tag="E", bufs=4)
                outb = wpool.tile([P, DOUT], F32, tag="outb", bufs=4)
                for half in range(2):
                    if pz_pre is not None:
                        pz = pz_pre[half]
                    else:
                        pz = psz.tile([P, DO // 2], F32, tag="pz")
                        for jj in range(2):
                            nsl_lo = 1024 * half + 512 * jj
                            nc.tensor.matmul(
                                pz[:, 512 * jj : 512 * (jj + 1)],
                                hT_a[:, P * s : P * (s + 1)],
                                w2a[:, nsl_lo : nsl_lo + 512],
                                start=True, stop=False,
                            )
                    for jj in range(2):
                        nsl_lo = 1024 * half + 512 * jj
                        nc.tensor.matmul(
                            pz[:, 512 * jj : 512 * (jj + 1)],
                            hT_b[:, P * s : P * (s + 1)],
                            w2b[:, nsl_lo : nsl_lo + 512],
                            start=False, stop=True,
                        )
                        if fine:
                            ste_q = (
                                steb[:, s, :]
                                .unsqueeze(1)
                                .broadcast_to([P, HALF_O // 2, ET])
                            )
                            eq = E[
                                :,
                                HALF_O * half + (HALF_O // 2) * jj :
                                HALF_O * half + (HALF_O // 2) * (jj + 1),
                            ]
                            nc.vector._custom_dve(
                                MULSCAN,
                                out=eq.unsqueeze(2).broadcast_to(
                                    [P, HALF_O // 2, ET]
                                ),
                                in0=pz[
                                    :, 512 * jj : 512 * (jj + 1)
                                ].rearrange("p (o d) -> p o d", d=ET),
                                in1=ste_q,
                            )
                    if not fine:
                        ste_b = (
                            steb[:, s, :]
                            .unsqueeze(1)
                            .broadcast_to([P, HALF_O, ET])
                        )
                        eslice = E[:, HALF_O * half : HALF_O * (half + 1)]
                        nc.vector._custom_dve(
                            MULSCAN,
                            out=eslice.unsqueeze(2).broadcast_to(
                                [P, HALF_O, ET]
                            ),
                            in0=pz[:].rearrange("p (o d) -> p o d", d=ET),
                            in1=ste_b,
                        )
                g = HALF_O // 2 if fine else HALF_O
                E_v = E[:].rearrange("p (h g) -> p h g", g=g)
                o_v = outb[:].rearrange("p (h g) -> p h g", g=g)
                nc.gpsimd.tensor_copy(o_v[:, :, 0:1], E_v[:, :, 0:1])
                nc.gpsimd.tensor_tensor(
                    o_v[:, :, 1:g],
                    E_v[:, :, 1:g],
                    E_v[:, :, 0 : g - 1],
                    op=mybir.AluOpType.subtract,
                )
                nc.scalar.dma_start(out_d[b, P * s : P * (s + 1), :], outb[:])

            # ---- pipeline: loads prefetched one batch ahead; batch b+1's
            # h phase is interleaved into batch b's z phase so the vector
            # engine (the z-phase pacer) never drains dry at batch
            # boundaries.
            loads = [emit_loads(0), emit_loads(1)]
            hT_a, h1n = emit_h01(0, *loads[0])
            # s=0 z 'a' halves early: keeps the PE streaming while the
            # scalar engine copies h1n/hT_b.
            pz_s0 = []
            for half in range(2):
                pz = psz.tile([P, DO // 2], F32, tag="pz")
                for jj in range(2):
                    nsl_lo = 1024 * half + 512 * jj
                    nc.tensor.matmul(
                        pz[:, 512 * jj : 512 * (jj + 1)],
                        hT_a[:, 0:P],
                        w2a[:, nsl_lo : nsl_lo + 512],
                        start=True, stop=False,
                    )
                pz_s0.append(pz)
            hT_b = emit_h2(0, loads[0][0], h1n)

            for b in range(B_PER_CORE):
                if b + 2 < B_PER_CORE:
                    loads.append(emit_loads(b + 2))
                nxt_a = nxt_n = nxt_b = None
                for s in range(S):
                    z_chunk(b, s, hT_a, hT_b, pz_s0 if s == 0 else None)
                    if b + 1 < B_PER_CORE:
                        if s == 1:
                            nxt_a, nxt_n = emit_h01(b + 1, *loads[b + 1])
                        elif s == 3:
                            nxt_b = emit_h2(b + 1, loads[b + 1][0], nxt_n)
                pz_s0 = None
                if b + 1 < B_PER_CORE:
                    hT_a, hT_b = nxt_a, nxt_b

    nc.compile()
    return nc


def _prep_consts(weights_pool, bias_pool):
    # W2 columns ordered (o, d): col = o*ET + d; rows ki = k*DIN + i.
    w2 = np.ascontiguousarray(
        weights_pool.transpose(1, 2, 3, 0).reshape(CHEB_K * DIN, DO)
    ).astype(np.float32)
    w2a = w2[:P].copy()
    w2a[:DIN] -= w2[2 * DIN :]  # h2 = 2 R@h1 - h0: fold -h0*W_k2 into W_k0
    w2b = np.concatenate(
        [2.0 * w2[P:], bias_pool.T.reshape(1, DO).astype(np.float32)], axis=0
    )
    i128 = np.eye(P, dtype=np.float32)
    bf = ml_dtypes.bfloat16
    return (
        np.ascontiguousarray(w2a).astype(bf),
        np.ascontiguousarray(w2b).astype(bf),
        i128.astype(bf),
    )


def kernel(x, STE, R, SC, weights_pool, bias_pool, _trace=False):
    x = np.asarray(x, dtype=np.float32)
    STE = np.asarray(STE, dtype=np.float32)
    R = np.asarray(R, dtype=np.float32)
    SC = np.asarray(SC, dtype=np.float32)
    weights_pool = np.asarray(weights_pool, dtype=np.float32)
    bias_pool = np.asarray(bias_pool, dtype=np.float32)

    if "nc" not in _cached:
        _cached["nc"] = _build_kernel()
    nc = _cached["nc"]

    bf = ml_dtypes.bfloat16
    w2a, w2b, i128 = _prep_consts(weights_pool, bias_pool)
    # Per-core R/SC shards with the contracted node axis m leading,
    # swizzled to [b, s, p, n] (m = s*128 + p) for linear stripe DMAs.
    RT_all = np.ascontiguousarray(
        R.transpose(0, 2, 1).reshape(B, S, P, N)
    ).astype(bf)
    SCT_all = np.ascontiguousarray(
        SC.transpose(0, 2, 1).reshape(B, S, P, N)
    ).astype(bf)
    in_maps = []
    for c in range(N_CORES):
        lo, hi = c * B_PER_CORE, (c + 1) * B_PER_CORE
        in_maps.append(
            {
                "RT": RT_all[lo:hi],
                "SCT": SCT_all[lo:hi],
                "x": np.ascontiguousarray(
                    x[lo:hi].reshape(B_PER_CORE, S, P, DIN).transpose(2, 0, 1, 3)
                ).astype(bf),
                "STE": np.ascontiguousarray(
                    STE[lo:hi].reshape(B_PER_CORE, S, P, ET).transpose(2, 0, 1, 3)
                ),
                "W2a": w2a,
                "W2b": w2b,
                "I128": i128,
            }
        )

    res = bass_utils.run_bass_kernel_spmd(
        nc, in_maps, core_ids=list(range(N_CORES)), trace=_trace
    )
    out = np.concatenate([r["out"] for r in res.results], axis=0)
    if _trace:
        kernel.last_result = res
    return out
